# revision 7
# baseline (speedup 1.0000x reference)
"""CompressedGLAHead Trainium2 kernel (v2: wavefront 8-lane scan).

Math (per batch element b, sequence of S tokens):
  q,k,v,alpha = Linear(x);  alpha = sigmoid(...)
  M[j] = Wd[:,j,:] @ Wu[j,:,:]                      (64 matrices, 128x128)
  b_t  = Wd @ vec(k_t v_t^T)
  c_t  = (sum_j alpha[t,j] M[j]) c_{t-1} + b_t      (sequential scan, d_c=128)
  o_t  = q_t^T reshape(Wu c_t, (64,64))

Structure (per core; core c handles batch b=c//2, half h=c%2):
  * 8 scan lanes per core, each 32 warmup + 128 real tokens (WIN=160,
    TOK=1280).  The scan is contractive, so zero-init + 32 warmup tokens
    reproduces the true state; warmup preceding t=0 uses a synthetic token
    x0 with W_k x0 + b_k = 0 (k=0 => b=0 => state stays 0).
  * Wavefront schedule: lane s executes scan-group g (32 tokens) during
    round r = s + g.  Within a round the 8 live lanes' steps interleave
    one-by-one, so the ~0.5us PE->engine->PE round trip per step hides
    behind the other lanes.  Lane readouts stagger (no serial tail).
  * Per-step state-writes rotate over Pool/DVE/Act; A-matrix PSUM->SBUF
    drains rotate over all three engines as well.
  * Projections run as [Wq|Wk] / [Walpha|Wv] 128-wide pairs (half the
    matmuls), drained by single Act activations with stacked biases.
  * kv outer products: DMA-replicated k rows * Pool-duplicated v, f16
    multiply on DVE; b_in and readout matmuls on PE as full-width f16.
"""

import numpy as np

import concourse.bass as bass
import concourse.tile as tile
from concourse import bacc
from concourse import mybir
from concourse.bass_utils import run_bass_kernel_spmd
from concourse.masks import make_identity

B, S, DM, DK, DV, DC = 4, 2048, 1024, 64, 64, 128
WARM = 32
LREAL = 128
NSUB = 8
WIN = WARM + LREAL          # 160
TOK = NSUB * WIN            # 1280
TG = 32                     # scan-group tokens
NG = WIN // TG              # 5 groups per lane
PG = 256                    # projection group tokens
NPG = TOK // PG             # 5
NROUND = NSUB + NG - 1      # 12 wavefront rounds

f32 = mybir.dt.float32
f32r = mybir.dt.float32r
f16 = mybir.dt.float16

_BUILT = {}


def _build_bass():
    nc = bacc.Bacc("TRN2", target_bir_lowering=False, debug=False)

    xsT = nc.dram_tensor("xsT", [DM, TOK], f32r, kind="ExternalInput")
    wqk = nc.dram_tensor("wqk", [DM, 128], f32r, kind="ExternalInput")
    wav = nc.dram_tensor("wav", [DM, 128], f32r, kind="ExternalInput")
    bqk = nc.dram_tensor("bqk", [128, 1], f32, kind="ExternalInput")
    bav = nc.dram_tensor("bav", [128, 1], f32, kind="ExternalInput")
    wdT = nc.dram_tensor("wdT", [DK * DV, DC], f16, kind="ExternalInput")
    wuT = nc.dram_tensor("wuT", [DC, DK * DV], f16, kind="ExternalInput")
    msb = nc.dram_tensor("msb", [DK, DC * DC], f16, kind="ExternalInput")
    o_out = nc.dram_tensor("o_out", [NSUB * LREAL, DV], f32, kind="ExternalOutput")

    with tile.TileContext(nc) as tc:
        _emit(nc, tc, xsT, wqk, wav, bqk, bav, wdT, wuT, msb, o_out)
    nc.compile()
    return nc


def _emit(nc, tc, xsT, wqk, wav, bqk, bav, wdT, wuT, msb, o_out):
    from contextlib import ExitStack

    add = mybir.AluOpType.add
    mult = mybir.AluOpType.mult
    ACT = mybir.ActivationFunctionType

    def _fence(ap):
        # 1-element ldweights on PE: absorbs one cross-engine wait so the
        # following matmul needs at most one (walrus LW sync-slot limit).
        if ap.dtype == f32:
            ap = ap.bitcast(f16)
        nc.tensor.ldweights(weights=ap)

    ctx = ExitStack()
    with ctx:
        consts = ctx.enter_context(tc.tile_pool(name="consts", bufs=1))

        # ---- resident weights (single consolidated DMAs) ----
        w_qk = consts.tile([128, 8, 128], f32r, name="w_qk")
        src = wqk[:, :]
        nc.sync.dma_start(
            out=w_qk,
            in_=bass.AP(tensor=src.tensor, offset=src.offset,
                        ap=[[128, 128], [128 * 128, 8], [1, 128]]))
        w_av = consts.tile([128, 8, 128], f32r, name="w_av")
        src = wav[:, :]
        nc.sync.dma_start(
            out=w_av,
            in_=bass.AP(tensor=src.tensor, offset=src.offset,
                        ap=[[128, 128], [128 * 128, 8], [1, 128]]))
        b_qk = consts.tile([128, 1], f32, name="b_qk")
        nc.sync.dma_start(out=b_qk, in_=bqk[:, :])
        b_av = consts.tile([128, 1], f32, name="b_av")
        nc.sync.dma_start(out=b_av, in_=bav[:, :])
        wdT_sb = consts.tile([128, 32, 128], f16)
        srcd = wdT[:, :]
        nc.sync.dma_start(
            out=wdT_sb,
            in_=bass.AP(tensor=srcd.tensor, offset=srcd.offset,
                        ap=[[128, 128], [128 * 128, 32], [1, 128]]))
        wuT_sb = consts.tile([128, 4096], f16)
        nc.sync.dma_start(out=wuT_sb, in_=wuT[:, :])
        msb_sb = consts.tile([64, DC * DC], f16)
        nc.sync.dma_start(out=msb_sb, in_=msb[:, :])
        ident = consts.tile([128, 128], f16)
        make_identity(nc, ident)

        # ---- persistent activations ----
        qkT = consts.tile([128, TOK], f16)   # rows 0-63 q, 64-127 k
        vaT = consts.tile([128, TOK], f16)   # rows 0-63 alpha, 64-127 v
        b_sb = consts.tile([128, TOK], f16)  # b_inT (c, t)
        cs = [consts.tile([128, WIN], f16, name=f"cs{s}") for s in range(NSUB)]
        q_sb = [consts.tile([128, 64], f32, name=f"q_sb{s}") for s in range(NSUB)]

        xp = ctx.enter_context(tc.tile_pool(name="xp", bufs=2))
        pp = ctx.enter_context(tc.tile_pool(name="pp", bufs=2, space="PSUM"))
        pb = ctx.enter_context(tc.tile_pool(name="pb", bufs=2, space="PSUM"))
        kvp = ctx.enter_context(tc.tile_pool(name="kvp", bufs=2))
        repp = ctx.enter_context(tc.tile_pool(name="repp", bufs=4))
        dupp = ctx.enter_context(tc.tile_pool(name="dupp", bufs=2))
        pa = ctx.enter_context(tc.tile_pool(name="pa", bufs=2, space="PSUM"))
        pc = ctx.enter_context(tc.tile_pool(name="pc", bufs=1, space="PSUM"))
        apool = ctx.enter_context(tc.tile_pool(name="apool", bufs=12))
        pt = ctx.enter_context(tc.tile_pool(name="pt", bufs=1, space="PSUM"))
        usbp = ctx.enter_context(tc.tile_pool(name="usbp", bufs=2))
        opool = ctx.enter_context(tc.tile_pool(name="opool", bufs=2))

        psCall = pc.tile([128, NSUB], f32, name="psCall")
        psC = [psCall[:, s:s + 1] for s in range(NSUB)]

        # xs chunk DMA (double buffered)
        xs_tiles = {}

        def load_xs(p):
            t = xp.tile([128, 8, PG], f32r, name="xs")
            src = xsT[:, :]
            nc.sync.dma_start(
                out=t,
                in_=bass.AP(tensor=src.tensor, offset=src.offset + p * PG,
                            ap=[[TOK, 128], [128 * TOK, 8], [1, PG]]))
            xs_tiles[p] = t

        # ---------------- P1 group: proj + kv + b_in ----------------
        def emit_p1(g):
            sl = slice(g * PG, (g + 1) * PG)
            xs = xs_tiles.pop(g)
            ps = pp.tile([128, 2, PG], f32, name="ps")
            if g > 0:
                _fence(qkT[0:1, g * PG - 1:g * PG])
            for s8 in range(8):
                nc.tensor.matmul(ps[:, 0, :], lhsT=w_qk[:, s8, :],
                                 rhs=xs[:, s8, :],
                                 start=(s8 == 0), stop=(s8 == 7))
            for s8 in range(8):
                nc.tensor.matmul(ps[:, 1, :], lhsT=w_av[:, s8, :],
                                 rhs=xs[:, s8, :],
                                 start=(s8 == 0), stop=(s8 == 7))
            nc.scalar.activation(out=qkT[:, sl], in_=ps[:, 0, :],
                                 func=ACT.Identity, bias=b_qk)
            nc.scalar.activation(out=vaT[0:64, sl], in_=ps[0:64, 1, :],
                                 func=ACT.Sigmoid, bias=b_av[0:64, :])
            nc.scalar.activation(out=vaT[64:128, sl], in_=ps[64:128, 1, :],
                                 func=ACT.Identity, bias=b_av[64:128, :])

            # kv outer products + b_in matmul
            vdup = dupp.tile([128, PG], f16)
            nc.gpsimd.tensor_copy(out=vdup[0:64, :], in_=vaT[64:128, sl])
            nc.gpsimd.tensor_copy(out=vdup[64:128, :], in_=vaT[64:128, sl])
            psb = pb.tile([128, PG], f32)
            if g == 0:
                _fence(wdT_sb[0:1, 0, 0:1])
            if g >= 2:
                _fence(b_sb[0:1, (g - 2) * PG:(g - 2) * PG + 1])
            for p in range(32):
                rep = repp.tile([128, PG], f16)
                src = qkT[64 + 2 * p:64 + 2 * p + 2, sl]
                rep_in = bass.AP(tensor=src.tensor, offset=src.offset,
                                 ap=[src.ap[0], [0, 64]] + src.ap[1:])
                nc.sync.dma_start(out=rep, in_=rep_in)
                kv = kvp.tile([128, PG], f16)
                nc.vector.tensor_tensor(out=kv, in0=rep, in1=vdup, op=mult)
                nc.tensor.matmul(psb, lhsT=wdT_sb[:, p, :], rhs=kv,
                                 start=(p == 0), stop=(p == 31))
            nc.vector.tensor_copy(out=b_sb[:, sl], in_=psb)

        # ---------------- A-precompute for lane s, group g -------------
        # returns the a_tile; drains rotate over engines
        drain_rot = [0]

        def _drain(out, in_):
            e = drain_rot[0] % 8
            drain_rot[0] += 1
            # weights: DVE 3, Act 3, Pool 2 out of 8
            if e in (0, 3, 6):
                nc.vector.tensor_copy(out=out, in_=in_)
            elif e in (1, 4, 7):
                nc.scalar.copy(out=out, in_=in_)
            else:
                nc.gpsimd.tensor_copy(out=out, in_=in_)

        def emit_apre(s, g):
            at = apool.tile([128, 128, TG], f16, name="at")
            t0 = s * WIN + g * TG
            chunks = []
            for bank in range(8):
                psa = pa.tile([128, 16 * TG], f32, name="psa")
                for ci in range(16):
                    cp = bank * 16 + ci
                    nc.tensor.matmul(
                        psa[:, ci * TG:(ci + 1) * TG],
                        lhsT=msb_sb[:, cp * 128:(cp + 1) * 128],
                        rhs=vaT[0:64, t0:t0 + TG],
                        start=True, stop=True)
                _drain(at[:, bank * 16:(bank + 1) * 16, :], psa)
            return at

        # ---------------- scan steps --------------------------------
        step_rot = [0]

        def emit_step(s, at, gt):
            # gt: 0-based token index within the lane window
            tcol = s * WIN + gt
            if gt == 0:
                nc.vector.tensor_copy(out=cs[s][:, 0:1],
                                      in_=b_sb[:, tcol:tcol + 1])
                return
            tl = gt % TG
            nc.tensor.matmul(psC[s], lhsT=at[:, :, tl],
                             rhs=cs[s][:, gt - 1:gt],
                             start=True, stop=True)
            e = step_rot[0] % 4
            step_rot[0] += 1
            if e == 0 or e == 2:
                nc.gpsimd.tensor_tensor(out=cs[s][:, gt:gt + 1],
                                        in0=psC[s],
                                        in1=b_sb[:, tcol:tcol + 1],
                                        op=add)
            elif e == 1:
                nc.vector.tensor_tensor(out=cs[s][:, gt:gt + 1],
                                        in0=psC[s],
                                        in1=b_sb[:, tcol:tcol + 1],
                                        op=add)
            else:
                nc.scalar.activation(out=cs[s][:, gt:gt + 1],
                                     in_=psC[s], func=ACT.Identity,
                                     bias=b_sb[:, tcol:tcol + 1])

        # ---------------- readout for lane s ------------------------
        stt_rot = [0]

        def emit_qtrans(s):
            lo = s * WIN + WARM
            pst = pt.tile([128, 64], f16)
            _fence(ident[0:1, 0:1])
            nc.tensor.transpose(out=pst, in_=qkT[0:64, lo:lo + 128],
                                identity=ident[0:64, 0:64])
            nc.scalar.copy(out=q_sb[s], in_=pst)

        def emit_readout(s):
            lo = WARM
            _fence(cs[s][0:1, lo:lo + 1])
            oa_v = opool.tile([128, 64], f32, name="oa_v")
            for sl8 in range(8):
                psu = pa.tile([128, 16 * TG], f32, name="psa")
                nc.tensor.matmul(psu, lhsT=cs[s][:, lo:lo + 128],
                                 rhs=wuT_sb[:, sl8 * 512:(sl8 + 1) * 512],
                                 start=True, stop=True)
                usb = usbp.tile([128, 512], f32, name="usb")
                nc.scalar.copy(out=usb, in_=psu)
                for jl in range(8):
                    j = sl8 * 8 + jl
                    qcol = q_sb[s][:, j:j + 1]
                    use_pool = (stt_rot[0] % 2 == 0)
                    stt_rot[0] += 1
                    eng = nc.gpsimd if use_pool else nc.vector
                    if j == 0:
                        eng.tensor_scalar_mul(
                            out=oa_v, in0=usb[:, jl * 64:(jl + 1) * 64],
                            scalar1=qcol)
                    else:
                        eng.scalar_tensor_tensor(
                            out=oa_v, in0=usb[:, jl * 64:(jl + 1) * 64],
                            scalar=qcol, in1=oa_v, op0=mult, op1=add)
            row0 = s * LREAL
            nc.sync.dma_start(out=o_out[row0:row0 + 128, :], in_=oa_v)

        # ================= schedule =================
        # P1 pacing: lane s group g needs tokens < 160s+32(g+1);
        # round r has max need col 160*min(7,r) + 32*(r-min(7,r)+1).
        p1_done = [0]

        def ensure_p1(col):
            while p1_done[0] * PG < col and p1_done[0] < NPG:
                if p1_done[0] + 1 < NPG and (p1_done[0] + 1) not in xs_tiles:
                    load_xs(p1_done[0] + 1)
                emit_p1(p1_done[0])
                p1_done[0] += 1

        load_xs(0)
        ensure_p1(1)            # group 0
        _fence(msb_sb[0:1, 0:1])
        _fence(wuT_sb[0:1, 0:1])

        # prologue: A-pre for lane 0 group 0
        a_cur = {}
        a_cur[0] = emit_apre(0, 0)

        for r in range(NROUND):
            lanes = [s for s in range(NSUB) if 0 <= r - s < NG]
            # P1 coverage: scan steps this round plus side A-pre reads
            need = 0
            for s in lanes:
                need = max(need, WIN * s + TG * (r - s + 2))
            if r + 1 < NSUB:
                need = max(need, WIN * (r + 1) + TG)
            ensure_p1(min(TOK, need))
            # PE side-work for this round: A-pre of each lane's next group,
            # plus the next lane's first group; plus readout of lane r-NG.
            side = []
            for s in lanes:
                g = r - s
                if g + 1 < NG:
                    side.append(("apre", s, g + 1))
            if r + 1 < NSUB:
                side.append(("apre", r + 1, 0))
            ro_lane = r - NG
            if 0 <= ro_lane < NSUB:
                side.append(("qtrans", ro_lane))
                side.append(("readout", ro_lane))

            # interleave: per step-slot emit one scan step per lane, then
            # a slice of side work.
            nside = len(side)
            done_side = 0
            a_next = {}
            for t in range(TG):
                for s in lanes:
                    g = r - s
                    emit_step(s, a_cur.get(s), g * TG + t)
                # emit side work spread over the first half of the round
                want = min(nside, (t + 1) * nside * 2 // TG)
                while done_side < want:
                    kind = side[done_side]
                    if kind[0] == "apre":
                        a_next[kind[1]] = emit_apre(kind[1], kind[2])
                    elif kind[0] == "qtrans":
                        emit_qtrans(kind[1])
                    else:
                        emit_readout(kind[1])
                    done_side += 1
            a_cur.update(a_next)

        # final readout for last lane
        emit_qtrans(NSUB - 1)
        emit_readout(NSUB - 1)


def _host_prep(inputs):
    x = np.asarray(inputs["x"], np.float32)
    Wq = np.asarray(inputs["W_q"], np.float32)
    Wk = np.asarray(inputs["W_k"], np.float32)
    Wv = np.asarray(inputs["W_v"], np.float32)
    Wa = np.asarray(inputs["W_alpha"], np.float32)
    bq = np.asarray(inputs["b_q"], np.float32)
    bk = np.asarray(inputs["b_k"], np.float32)
    bv = np.asarray(inputs["b_v"], np.float32)
    ba = np.asarray(inputs["b_alpha"], np.float32)
    x0 = np.linalg.lstsq(Wk.astype(np.float64), -bk.astype(np.float64),
                         rcond=None)[0].astype(np.float32)
    shared = {
        "wqk": np.ascontiguousarray(np.concatenate([Wq, Wk], 0).T),
        "wav": np.ascontiguousarray(np.concatenate([Wa, Wv], 0).T),
        "bqk": np.concatenate([bq, bk]).reshape(128, 1),
        "bav": np.concatenate([ba, bv]).reshape(128, 1),
        "wdT": np.ascontiguousarray(
            np.asarray(inputs["W_down"], np.float32).T).astype(np.float16),
        "wuT": np.ascontiguousarray(
            np.asarray(inputs["W_up"], np.float32).T).astype(np.float16),
        "msb": np.ascontiguousarray(
            np.einsum("cjv,jvd->jcd",
                      np.asarray(inputs["W_down"], np.float32).reshape(DC, DK, DV),
                      np.asarray(inputs["W_up"], np.float32).reshape(DK, DV, DC)
                      ).reshape(DK, DC * DC)).astype(np.float16),
    }
    in_maps = []
    for core in range(8):
        b, h = core // 2, core % 2
        base = h * 1024
        segs = []
        for i in range(NSUB):
            lo = base + i * LREAL - WARM
            hi = base + i * LREAL + LREAL
            if lo < 0:
                seg = np.concatenate([np.tile(x0, (-lo, 1)), x[b, 0:hi]], axis=0)
            else:
                seg = x[b, lo:hi]
            segs.append(seg)
        xs = np.concatenate(segs, axis=0)          # (1280, 1024)
        m = dict(shared)
        m["xsT"] = np.ascontiguousarray(xs.T)      # (1024, 1280)
        in_maps.append(m)
    return in_maps


def kernel(**inputs):
    if "nc" not in _BUILT:
        _BUILT["nc"] = _build_bass()
    nc = _BUILT["nc"]
    in_maps = _host_prep(inputs)
    res = run_bass_kernel_spmd(nc, in_maps, core_ids=list(range(8)))
    results = res.results if hasattr(res, "results") else res
    o = np.zeros((B, S, DV), np.float32)
    for core in range(8):
        b, h = core // 2, core % 2
        o[b, h * 1024:(h + 1) * 1024, :] = results[core]["o_out"]
    return o


# revision 10
# speedup vs baseline: 1.6305x; 1.6305x over previous
"""CompressedGLAHead Trainium2 kernel (v2: wavefront 8-lane scan).

Math (per batch element b, sequence of S tokens):
  q,k,v,alpha = Linear(x);  alpha = sigmoid(...)
  M[j] = Wd[:,j,:] @ Wu[j,:,:]                      (64 matrices, 128x128)
  b_t  = Wd @ vec(k_t v_t^T)
  c_t  = (sum_j alpha[t,j] M[j]) c_{t-1} + b_t      (sequential scan, d_c=128)
  o_t  = q_t^T reshape(Wu c_t, (64,64))

Structure (per core; core c handles batch b=c//2, half h=c%2):
  * 8 scan lanes per core, each 32 warmup + 128 real tokens (WIN=160,
    TOK=1280).  The scan is contractive, so zero-init + 32 warmup tokens
    reproduces the true state; warmup preceding t=0 uses a synthetic token
    x0 with W_k x0 + b_k = 0 (k=0 => b=0 => state stays 0).
  * Wavefront schedule: lane s executes scan-group g (32 tokens) during
    round r = s + g.  Within a round the 8 live lanes' steps interleave
    one-by-one, so the ~0.5us PE->engine->PE round trip per step hides
    behind the other lanes.  Lane readouts stagger (no serial tail).
  * Per-step state-writes rotate over Pool/DVE/Act; A-matrix PSUM->SBUF
    drains rotate over all three engines as well.
  * Projections run as [Wq|Wk] / [Walpha|Wv] 128-wide pairs (half the
    matmuls), drained by single Act activations with stacked biases.
  * kv outer products: DMA-replicated k rows * Pool-duplicated v, f16
    multiply on DVE; b_in and readout matmuls on PE as full-width f16.
"""

import numpy as np

import concourse.bass as bass
import concourse.tile as tile
from concourse import bacc
from concourse import mybir
from concourse.bass_utils import run_bass_kernel_spmd
from concourse.masks import make_identity

B, S, DM, DK, DV, DC = 4, 2048, 1024, 64, 64, 128
WARM = 32
LREAL = 128
NSUB = 8
WIN = WARM + LREAL          # 160
TOK = NSUB * WIN            # 1280
TG = 32                     # scan-group tokens
NG = WIN // TG              # 5 groups per lane
PG = 256                    # projection group tokens
NPG = TOK // PG             # 5
NROUND = NSUB + NG - 1      # 12 wavefront rounds

f32 = mybir.dt.float32
f32r = mybir.dt.float32r
f16 = mybir.dt.float16

_BUILT = {}


def _build_bass():
    nc = bacc.Bacc("TRN2", target_bir_lowering=False, debug=False)

    xsT = nc.dram_tensor("xsT", [DM, TOK], f32r, kind="ExternalInput")
    wqk = nc.dram_tensor("wqk", [DM, 128], f32r, kind="ExternalInput")
    wav = nc.dram_tensor("wav", [DM, 128], f32r, kind="ExternalInput")
    bqk = nc.dram_tensor("bqk", [128, 1], f32, kind="ExternalInput")
    bav = nc.dram_tensor("bav", [128, 1], f32, kind="ExternalInput")
    wdT = nc.dram_tensor("wdT", [DK * DV, DC], f16, kind="ExternalInput")
    wuT = nc.dram_tensor("wuT", [DC, DK * DV], f16, kind="ExternalInput")
    msb = nc.dram_tensor("msb", [DK, DC * DC], f16, kind="ExternalInput")
    o_out = nc.dram_tensor("o_out", [NSUB * LREAL, DV], f32, kind="ExternalOutput")

    with tile.TileContext(nc) as tc:
        _emit(nc, tc, xsT, wqk, wav, bqk, bav, wdT, wuT, msb, o_out)
    nc.compile()
    return nc


def _emit(nc, tc, xsT, wqk, wav, bqk, bav, wdT, wuT, msb, o_out):
    from contextlib import ExitStack

    add = mybir.AluOpType.add
    mult = mybir.AluOpType.mult
    ACT = mybir.ActivationFunctionType

    def _fence(ap):
        # 1-element ldweights on PE: absorbs one cross-engine wait so the
        # following matmul needs at most one (walrus LW sync-slot limit).
        if ap.dtype == f32:
            ap = ap.bitcast(f16)
        nc.tensor.ldweights(weights=ap)

    ctx = ExitStack()
    with ctx:
        consts = ctx.enter_context(tc.tile_pool(name="consts", bufs=1))

        # ---- resident weights (single consolidated DMAs) ----
        w_qk = consts.tile([128, 8, 128], f32r, name="w_qk")
        src = wqk[:, :]
        nc.sync.dma_start(
            out=w_qk,
            in_=bass.AP(tensor=src.tensor, offset=src.offset,
                        ap=[[128, 128], [128 * 128, 8], [1, 128]]))
        w_av = consts.tile([128, 8, 128], f32r, name="w_av")
        src = wav[:, :]
        nc.sync.dma_start(
            out=w_av,
            in_=bass.AP(tensor=src.tensor, offset=src.offset,
                        ap=[[128, 128], [128 * 128, 8], [1, 128]]))
        b_qk = consts.tile([128, 1], f32, name="b_qk")
        nc.sync.dma_start(out=b_qk, in_=bqk[:, :])
        b_av = consts.tile([128, 1], f32, name="b_av")
        nc.sync.dma_start(out=b_av, in_=bav[:, :])
        wdT_sb = consts.tile([128, 32, 128], f16)
        srcd = wdT[:, :]
        nc.sync.dma_start(
            out=wdT_sb,
            in_=bass.AP(tensor=srcd.tensor, offset=srcd.offset,
                        ap=[[128, 128], [128 * 128, 32], [1, 128]]))
        wuT_sb = consts.tile([128, 4096], f16)
        nc.sync.dma_start(out=wuT_sb, in_=wuT[:, :])
        msb_sb = consts.tile([64, DC * DC], f16)
        nc.sync.dma_start(out=msb_sb, in_=msb[:, :])
        ident = consts.tile([128, 128], f16)
        make_identity(nc, ident)

        # ---- persistent activations ----
        qkT = consts.tile([128, TOK], f16)   # rows 0-63 q, 64-127 k
        vaT = consts.tile([128, TOK], f16)   # rows 0-63 alpha, 64-127 v
        b_sb = consts.tile([128, TOK], f16)  # b_inT (c, t)
        cs = [consts.tile([128, WIN], f16, name=f"cs{s}") for s in range(NSUB)]
        q_sb = [consts.tile([128, 64], f32, name=f"q_sb{s}") for s in range(NSUB)]

        xp = ctx.enter_context(tc.tile_pool(name="xp", bufs=2))
        pp = ctx.enter_context(tc.tile_pool(name="pp", bufs=1, space="PSUM"))
        pb = ctx.enter_context(tc.tile_pool(name="pb", bufs=1, space="PSUM"))
        kvp = ctx.enter_context(tc.tile_pool(name="kvp", bufs=2))
        repp = ctx.enter_context(tc.tile_pool(name="repp", bufs=4))
        dupp = ctx.enter_context(tc.tile_pool(name="dupp", bufs=2))
        pa = ctx.enter_context(tc.tile_pool(name="pa", bufs=2, space="PSUM"))
        pc = ctx.enter_context(tc.tile_pool(name="pc", bufs=1, space="PSUM"))
        apool = ctx.enter_context(tc.tile_pool(name="apool", bufs=12))
        usbp = ctx.enter_context(tc.tile_pool(name="usbp", bufs=2))
        opool = ctx.enter_context(tc.tile_pool(name="opool", bufs=2))

        # one PSUM bank per lane-pair: lanes s and s+4 share a tile in
        # disjoint columns
        psCt = [pc.tile([128, 2], f32, name=f"psC{i}") for i in range(4)]
        psC = [psCt[s % 4][:, s // 4:s // 4 + 1] for s in range(NSUB)]

        # xs chunk DMA (double buffered)
        xs_tiles = {}

        def load_xs(p):
            t = xp.tile([128, 8, PG], f32r, name="xs")
            src = xsT[:, :]
            nc.sync.dma_start(
                out=t,
                in_=bass.AP(tensor=src.tensor, offset=src.offset + p * PG,
                            ap=[[TOK, 128], [128 * TOK, 8], [1, PG]]))
            xs_tiles[p] = t

        # ---------------- P1 group: proj + kv + b_in ----------------
        def emit_p1(g):
            sl = slice(g * PG, (g + 1) * PG)
            xs = xs_tiles.pop(g)
            ps = pp.tile([128, 2, PG], f32, name="ps")
            if g > 0:
                _fence(qkT[0:1, g * PG - 1:g * PG])
            for s8 in range(8):
                nc.tensor.matmul(ps[:, 0, :], lhsT=w_qk[:, s8, :],
                                 rhs=xs[:, s8, :],
                                 start=(s8 == 0), stop=(s8 == 7))
            for s8 in range(8):
                nc.tensor.matmul(ps[:, 1, :], lhsT=w_av[:, s8, :],
                                 rhs=xs[:, s8, :],
                                 start=(s8 == 0), stop=(s8 == 7))
            nc.scalar.activation(out=qkT[:, sl], in_=ps[:, 0, :],
                                 func=ACT.Identity, bias=b_qk)
            nc.scalar.activation(out=vaT[0:64, sl], in_=ps[0:64, 1, :],
                                 func=ACT.Sigmoid, bias=b_av[0:64, :])
            nc.scalar.activation(out=vaT[64:128, sl], in_=ps[64:128, 1, :],
                                 func=ACT.Identity, bias=b_av[64:128, :])

            # kv outer products + b_in matmul
            vdup = dupp.tile([128, PG], f16)
            nc.gpsimd.tensor_copy(out=vdup[0:64, :], in_=vaT[64:128, sl])
            nc.gpsimd.tensor_copy(out=vdup[64:128, :], in_=vaT[64:128, sl])
            psb = pb.tile([128, PG], f32)
            if g == 0:
                _fence(wdT_sb[0:1, 0, 0:1])
            if g >= 2:
                _fence(b_sb[0:1, (g - 2) * PG:(g - 2) * PG + 1])
            for p in range(32):
                rep = repp.tile([128, PG], f16)
                src = qkT[64 + 2 * p:64 + 2 * p + 2, sl]
                rep_in = bass.AP(tensor=src.tensor, offset=src.offset,
                                 ap=[src.ap[0], [0, 64]] + src.ap[1:])
                nc.sync.dma_start(out=rep, in_=rep_in)
                kv = kvp.tile([128, PG], f16)
                nc.vector.tensor_tensor(out=kv, in0=rep, in1=vdup, op=mult)
                nc.tensor.matmul(psb, lhsT=wdT_sb[:, p, :], rhs=kv,
                                 start=(p == 0), stop=(p == 31))
            nc.vector.tensor_copy(out=b_sb[:, sl], in_=psb)

        # ---------------- A-precompute for lane s, group g -------------
        # returns the a_tile; drains rotate over engines
        drain_rot = [0]

        def _drain(out, in_):
            e = drain_rot[0] % 8
            drain_rot[0] += 1
            # weights: DVE 3, Act 3, Pool 2 out of 8
            if e in (0, 3, 6):
                nc.vector.tensor_copy(out=out, in_=in_)
            elif e in (1, 4, 7):
                nc.scalar.copy(out=out, in_=in_)
            else:
                nc.gpsimd.tensor_copy(out=out, in_=in_)

        def emit_apre(s, g):
            at = apool.tile([128, 128, TG], f16, name="at")
            t0 = s * WIN + g * TG
            chunks = []
            for bank in range(8):
                psa = pa.tile([128, 16 * TG], f32, name="psa")
                for ci in range(16):
                    cp = bank * 16 + ci
                    nc.tensor.matmul(
                        psa[:, ci * TG:(ci + 1) * TG],
                        lhsT=msb_sb[:, cp * 128:(cp + 1) * 128],
                        rhs=vaT[0:64, t0:t0 + TG],
                        start=True, stop=True)
                _drain(at[:, bank * 16:(bank + 1) * 16, :], psa)
            return at

        # ---------------- scan steps --------------------------------
        step_rot = [0]

        def emit_step(s, at, gt):
            # gt: 0-based token index within the lane window
            tcol = s * WIN + gt
            if gt == 0:
                nc.vector.tensor_copy(out=cs[s][:, 0:1],
                                      in_=b_sb[:, tcol:tcol + 1])
                return
            tl = gt % TG
            nc.tensor.matmul(psC[s], lhsT=at[:, :, tl],
                             rhs=cs[s][:, gt - 1:gt],
                             start=True, stop=True)
            e = step_rot[0] % 4
            step_rot[0] += 1
            if e == 0 or e == 2:
                nc.gpsimd.tensor_tensor(out=cs[s][:, gt:gt + 1],
                                        in0=psC[s],
                                        in1=b_sb[:, tcol:tcol + 1],
                                        op=add)
            elif e == 1:
                nc.vector.tensor_tensor(out=cs[s][:, gt:gt + 1],
                                        in0=psC[s],
                                        in1=b_sb[:, tcol:tcol + 1],
                                        op=add)
            else:
                nc.scalar.activation(out=cs[s][:, gt:gt + 1],
                                     in_=psC[s], func=ACT.Identity,
                                     bias=b_sb[:, tcol:tcol + 1])

        # ---------------- readout for lane s ------------------------
        stt_rot = [0]

        def emit_qtrans(s):
            lo = s * WIN + WARM
            pst = pa.tile([128, 16 * TG], f32, name="psa")
            pst16 = pst.bitcast(f16)[:, 0:64]
            _fence(ident[0:1, 0:1])
            nc.tensor.transpose(out=pst16, in_=qkT[0:64, lo:lo + 128],
                                identity=ident[0:64, 0:64])
            nc.scalar.copy(out=q_sb[s], in_=pst16)

        def emit_readout(s):
            lo = WARM
            _fence(cs[s][0:1, lo:lo + 1])
            oa_v = opool.tile([128, 64], f32, name="oa_v")
            for sl8 in range(8):
                psu = pa.tile([128, 16 * TG], f32, name="psa")
                nc.tensor.matmul(psu, lhsT=cs[s][:, lo:lo + 128],
                                 rhs=wuT_sb[:, sl8 * 512:(sl8 + 1) * 512],
                                 start=True, stop=True)
                usb = usbp.tile([128, 512], f32, name="usb")
                nc.scalar.copy(out=usb, in_=psu)
                for jl in range(8):
                    j = sl8 * 8 + jl
                    qcol = q_sb[s][:, j:j + 1]
                    use_pool = (stt_rot[0] % 2 == 0)
                    stt_rot[0] += 1
                    eng = nc.gpsimd if use_pool else nc.vector
                    if j == 0:
                        eng.tensor_scalar_mul(
                            out=oa_v, in0=usb[:, jl * 64:(jl + 1) * 64],
                            scalar1=qcol)
                    else:
                        eng.scalar_tensor_tensor(
                            out=oa_v, in0=usb[:, jl * 64:(jl + 1) * 64],
                            scalar=qcol, in1=oa_v, op0=mult, op1=add)
            row0 = s * LREAL
            nc.sync.dma_start(out=o_out[row0:row0 + 128, :], in_=oa_v)

        # ================= schedule =================
        # P1 pacing: lane s group g needs tokens < 160s+32(g+1);
        # round r has max need col 160*min(7,r) + 32*(r-min(7,r)+1).
        p1_done = [0]

        def ensure_p1(col):
            while p1_done[0] * PG < col and p1_done[0] < NPG:
                if p1_done[0] + 1 < NPG and (p1_done[0] + 1) not in xs_tiles:
                    load_xs(p1_done[0] + 1)
                emit_p1(p1_done[0])
                p1_done[0] += 1

        load_xs(0)
        ensure_p1(1)            # group 0
        _fence(msb_sb[0:1, 0:1])
        _fence(wuT_sb[0:1, 0:1])

        # prologue: A-pre for lane 0 group 0
        a_cur = {}
        a_cur[0] = emit_apre(0, 0)

        for r in range(NROUND):
            lanes = [s for s in range(NSUB) if 0 <= r - s < NG]
            # P1 coverage: scan steps this round plus side A-pre reads
            need = 0
            for s in lanes:
                need = max(need, WIN * s + TG * (r - s + 2))
            if r + 1 < NSUB:
                need = max(need, WIN * (r + 1) + TG)
            ensure_p1(min(TOK, need))
            # PE side-work for this round: A-pre of each lane's next group,
            # plus the next lane's first group; plus readout of lane r-NG.
            side = []
            for s in lanes:
                g = r - s
                if g + 1 < NG:
                    side.append(("apre", s, g + 1))
            if r + 1 < NSUB:
                side.append(("apre", r + 1, 0))
            ro_lane = r - NG
            if 0 <= ro_lane < NSUB:
                side.append(("qtrans", ro_lane))
                side.append(("readout", ro_lane))

            # interleave: per step-slot emit one scan step per lane, then
            # a slice of side work.
            nside = len(side)
            done_side = 0
            a_next = {}
            for t in range(TG):
                for s in lanes:
                    g = r - s
                    emit_step(s, a_cur.get(s), g * TG + t)
                # emit side work spread over the first half of the round
                want = min(nside, (t + 1) * nside * 2 // TG)
                while done_side < want:
                    kind = side[done_side]
                    if kind[0] == "apre":
                        a_next[kind[1]] = emit_apre(kind[1], kind[2])
                    elif kind[0] == "qtrans":
                        emit_qtrans(kind[1])
                    else:
                        emit_readout(kind[1])
                    done_side += 1
            a_cur.update(a_next)

        # final readout for last lane
        emit_qtrans(NSUB - 1)
        emit_readout(NSUB - 1)


def _host_prep(inputs):
    x = np.asarray(inputs["x"], np.float32)
    Wq = np.asarray(inputs["W_q"], np.float32)
    Wk = np.asarray(inputs["W_k"], np.float32)
    Wv = np.asarray(inputs["W_v"], np.float32)
    Wa = np.asarray(inputs["W_alpha"], np.float32)
    bq = np.asarray(inputs["b_q"], np.float32)
    bk = np.asarray(inputs["b_k"], np.float32)
    bv = np.asarray(inputs["b_v"], np.float32)
    ba = np.asarray(inputs["b_alpha"], np.float32)
    x0 = np.linalg.lstsq(Wk.astype(np.float64), -bk.astype(np.float64),
                         rcond=None)[0].astype(np.float32)
    shared = {
        "wqk": np.ascontiguousarray(np.concatenate([Wq, Wk], 0).T),
        "wav": np.ascontiguousarray(np.concatenate([Wa, Wv], 0).T),
        "bqk": np.concatenate([bq, bk]).reshape(128, 1),
        "bav": np.concatenate([ba, bv]).reshape(128, 1),
        "wdT": np.ascontiguousarray(
            np.asarray(inputs["W_down"], np.float32).T).astype(np.float16),
        "wuT": np.ascontiguousarray(
            np.asarray(inputs["W_up"], np.float32).T).astype(np.float16),
        "msb": np.ascontiguousarray(
            np.einsum("cjv,jvd->jcd",
                      np.asarray(inputs["W_down"], np.float32).reshape(DC, DK, DV),
                      np.asarray(inputs["W_up"], np.float32).reshape(DK, DV, DC)
                      ).reshape(DK, DC * DC)).astype(np.float16),
    }
    in_maps = []
    for core in range(8):
        b, h = core // 2, core % 2
        base = h * 1024
        segs = []
        for i in range(NSUB):
            lo = base + i * LREAL - WARM
            hi = base + i * LREAL + LREAL
            if lo < 0:
                seg = np.concatenate([np.tile(x0, (-lo, 1)), x[b, 0:hi]], axis=0)
            else:
                seg = x[b, lo:hi]
            segs.append(seg)
        xs = np.concatenate(segs, axis=0)          # (1280, 1024)
        m = dict(shared)
        m["xsT"] = np.ascontiguousarray(xs.T)      # (1024, 1280)
        in_maps.append(m)
    return in_maps


def kernel(**inputs):
    if "nc" not in _BUILT:
        _BUILT["nc"] = _build_bass()
    nc = _BUILT["nc"]
    in_maps = _host_prep(inputs)
    res = run_bass_kernel_spmd(nc, in_maps, core_ids=list(range(8)))
    results = res.results if hasattr(res, "results") else res
    o = np.zeros((B, S, DV), np.float32)
    for core in range(8):
        b, h = core // 2, core % 2
        o[b, h * 1024:(h + 1) * 1024, :] = results[core]["o_out"]
    return o


# revision 39
# speedup vs baseline: 1.7135x; 1.0509x over previous
"""CompressedGLAHead Trainium2 kernel (v3: factored token-major scan).

Math (per batch element b, sequence of S tokens):
  q,k,v,alpha = Linear(x);  alpha = sigmoid(...)
  b_t  = Wd @ vec(k_t v_t^T)
  c_t  = A_t c_{t-1} + b_t,  A_t = Wd diag(rep(alpha_t)) Wu
  o_t  = q_t^T reshape(Wu c_t, (64,64))

Key structure:
  * The A_t matvec is FACTORED per step: u = Wu c (32 matmuls), gate
    u *= alpha (one DVE + one Pool half), c' = Wd gate(u) + b (32
    accumulating matmuls + one engine add).  No per-token A matrices are
    materialized, eliminating the PSUM->SBUF drain traffic that
    dominates an A-materialized design.
  * 16 scan lanes (32 warmup + 64 real tokens each; contractive scan =>
    zero-init + 32-token warmup reproduces the true state; tokens before
    t=0 use a synthetic x0 with W_k x0 + b_k = 0 so the warm state stays
    exactly 0).  The lane streams are interleaved TOKEN-MAJOR: stream
    column gt*16+s holds lane s's token gt.  All 16 lanes advance in one
    step: matmul rhs/state-write/gate operands are all contiguous
    (128, 16) slices.
  * kv chunks are v-major with 4-row v replication (16 rep DMAs per
    projection group) and k duplicated by engine copies; Wd/Wu are
    host-permuted to match.
  * Readout (after the scan): per lane-pair window, psu = cs^T Wu on
    PE, Act drains, q-weighted reduction as 8 parallel stt chains + a
    pairwise add tree on DVE/Pool.
"""

import numpy as np

import concourse.bass as bass
import concourse.tile as tile
from concourse import bacc
from concourse import mybir
from concourse.bass_utils import run_bass_kernel_spmd
from concourse.masks import make_identity

B, S, DM, DK, DV, DC = 4, 2048, 1024, 64, 64, 128
WARM = 32
LREAL = 64
NSUB = 16
WIN = WARM + LREAL          # 96
TOK = NSUB * WIN            # 1536
PGT = 16                    # projection group: 16 token-steps = 256 cols
PG = PGT * NSUB             # 256
NPG = TOK // PG             # 6
NPAIR = NSUB // 2           # 8 readout pair-windows

f32 = mybir.dt.float32
f32r = mybir.dt.float32r
f16 = mybir.dt.float16

_BUILT = {}
STEP_MAP = {}


def _build_bass():
    nc = bacc.Bacc("TRN2", target_bir_lowering=False, debug=False)

    xsT = nc.dram_tensor("xsT", [DM, TOK], f32r, kind="ExternalInput")
    wqk = nc.dram_tensor("wqk", [DM, 128], f32r, kind="ExternalInput")
    wav = nc.dram_tensor("wav", [DM, 128], f32r, kind="ExternalInput")
    bqk = nc.dram_tensor("bqk", [128, 1], f32, kind="ExternalInput")
    bav = nc.dram_tensor("bav", [128, 1], f32, kind="ExternalInput")
    # v-major-permuted Wd/Wu for the scan; k-major Wu for the readout
    wdvm = nc.dram_tensor("wdvm", [DK * DV, DC], f16, kind="ExternalInput")
    wuvm = nc.dram_tensor("wuvm", [DC, DK * DV], f16, kind="ExternalInput")
    wukm = nc.dram_tensor("wukm", [DC, DK * DV], f16, kind="ExternalInput")
    o_out = nc.dram_tensor("o_out", [NSUB * LREAL, DV], f32, kind="ExternalOutput")

    with tile.TileContext(nc) as tc:
        _emit(nc, tc, xsT, wqk, wav, bqk, bav, wdvm, wuvm, wukm, o_out)
    nc.compile()
    return nc


def _emit(nc, tc, xsT, wqk, wav, bqk, bav, wdvm, wuvm, wukm, o_out):
    from contextlib import ExitStack

    add = mybir.AluOpType.add
    mult = mybir.AluOpType.mult
    ACT = mybir.ActivationFunctionType

    def _fence(ap):
        if ap.dtype == f32:
            ap = ap.bitcast(f16)
        nc.tensor.ldweights(weights=ap)

    ctx = ExitStack()
    with ctx:
        consts = ctx.enter_context(tc.tile_pool(name="consts", bufs=1))

        # ---- resident weights ----
        w_qk = consts.tile([128, 8, 128], f32r, name="w_qk")
        src = wqk[:, :]
        nc.sync.dma_start(
            out=w_qk,
            in_=bass.AP(tensor=src.tensor, offset=src.offset,
                        ap=[[128, 128], [128 * 128, 8], [1, 128]]))
        w_av = consts.tile([128, 8, 128], f32r, name="w_av")
        src = wav[:, :]
        nc.sync.dma_start(
            out=w_av,
            in_=bass.AP(tensor=src.tensor, offset=src.offset,
                        ap=[[128, 128], [128 * 128, 8], [1, 128]]))
        b_qk = consts.tile([128, 1], f32, name="b_qk")
        nc.sync.dma_start(out=b_qk, in_=bqk[:, :])
        b_av = consts.tile([128, 1], f32, name="b_av")
        nc.sync.dma_start(out=b_av, in_=bav[:, :])
        wd_sb = consts.tile([128, 32, 128], f16, name="wd_sb")
        srcd = wdvm[:, :]
        nc.sync.dma_start(
            out=wd_sb,
            in_=bass.AP(tensor=srcd.tensor, offset=srcd.offset,
                        ap=[[128, 128], [128 * 128, 32], [1, 128]]))
        wu_sb = consts.tile([128, 32, 128], f16, name="wu_sb")
        nc.sync.dma_start(out=wu_sb, in_=wuvm[:, :])
        wukm_sb = consts.tile([128, 4096], f16, name="wukm_sb")
        nc.sync.dma_start(out=wukm_sb, in_=wukm[:, :])
        ident = consts.tile([128, 128], f16)
        make_identity(nc, ident)

        # ---- persistent activations ----
        qkT = consts.tile([128, TOK], f16)    # rows 0-63 q, 64-127 k
        vaT = consts.tile([128, TOK], f16)    # rows 0-63 alpha, 64-127 v
        adup = consts.tile([128, TOK], f16)   # [alpha; alpha]
        b_sb = consts.tile([128, TOK], f16)   # b_inT (c, col)
        cs_all = consts.tile([128, WIN, NSUB], f16, name="cs_all")
        gt_sb = consts.tile([128, 32, NSUB], f16, name="gt_sb")  # gated u
        q_sb = [consts.tile([128, 64], f32, name=f"q_sb{i}") for i in range(2)]

        xp = ctx.enter_context(tc.tile_pool(name="xp", bufs=2))
        pp = ctx.enter_context(tc.tile_pool(name="pp", bufs=2, space="PSUM"))
        pb = ctx.enter_context(tc.tile_pool(name="pb", bufs=2, space="PSUM"))
        pu = ctx.enter_context(tc.tile_pool(name="pu", bufs=1, space="PSUM"))
        pcp = ctx.enter_context(tc.tile_pool(name="pcp", bufs=1, space="PSUM"))
        kvp = ctx.enter_context(tc.tile_pool(name="kvp", bufs=2))
        repp = ctx.enter_context(tc.tile_pool(name="repp", bufs=4))
        dupp = ctx.enter_context(tc.tile_pool(name="dupp", bufs=2))
        usbp = ctx.enter_context(tc.tile_pool(name="usbp", bufs=2))
        opool = ctx.enter_context(tc.tile_pool(name="opool", bufs=4))

        psU = pu.tile([128, 32, NSUB], f32, name="psU")
        psC = pcp.tile([128, NSUB], f32, name="psC")

        xs_tiles = {}

        def load_xs(p):
            t = xp.tile([128, 8, PG], f32r, name="xs")
            src = xsT[:, :]
            nc.sync.dma_start(
                out=t,
                in_=bass.AP(tensor=src.tensor, offset=src.offset + p * PG,
                            ap=[[TOK, 128], [128 * TOK, 8], [1, PG]]))
            xs_tiles[p] = t

        # ---------------- P1 group: proj + kv + b_in ----------------
        def emit_p1(g):
            sl = slice(g * PG, (g + 1) * PG)
            xs = xs_tiles.pop(g)
            ps = pp.tile([128, 512], f32, name="ps")
            ps_qk, ps_va = ps[:, 0:PG], ps[:, PG:2 * PG]
            if g > 0:
                _fence(qkT[0:1, g * PG - 1:g * PG])
            for s8 in range(8):
                nc.tensor.matmul(ps_qk, lhsT=w_qk[:, s8, :],
                                 rhs=xs[:, s8, :],
                                 start=(s8 == 0), stop=(s8 == 7))
            for s8 in range(8):
                nc.tensor.matmul(ps_va, lhsT=w_av[:, s8, :],
                                 rhs=xs[:, s8, :],
                                 start=(s8 == 0), stop=(s8 == 7))
            nc.scalar.activation(out=qkT[:, sl], in_=ps_qk,
                                 func=ACT.Identity, bias=b_qk)
            nc.scalar.activation(out=vaT[0:64, sl], in_=ps_va[0:64, :],
                                 func=ACT.Sigmoid, bias=b_av[0:64, :])
            nc.scalar.activation(out=vaT[64:128, sl], in_=ps_va[64:128, :],
                                 func=ACT.Identity, bias=b_av[64:128, :])
            # alpha duplicated for the gate ops
            nc.vector.tensor_copy(out=adup[0:64, sl], in_=vaT[0:64, sl])
            nc.gpsimd.tensor_copy(out=adup[64:128, sl], in_=vaT[0:64, sl])

            # kdup: [k; k]
            kd = dupp.tile([128, PG], f16, name="kd")
            nc.gpsimd.tensor_copy(out=kd[0:64, :], in_=qkT[64:128, sl])
            nc.vector.tensor_copy(out=kd[64:128, :], in_=qkT[64:128, sl])
            psb = pb.tile([128, PG], f32, name="psb")
            if g == 0:
                _fence(wd_sb[0:1, 0, 0:1])
            if g >= 2:
                _fence(b_sb[0:1, (g - 2) * PG:(g - 2) * PG + 1])
            # v-major chunk p: rows r -> k=r%64, v=2p+r//64
            for p in range(32):
                rep = repp.tile([128, PG], f16)
                srcv = vaT[64 + 2 * p:64 + 2 * p + 2, sl]
                rep_in = bass.AP(tensor=srcv.tensor, offset=srcv.offset,
                                 ap=[srcv.ap[0], [0, 64]] + list(srcv.ap[1:]))
                nc.sync.dma_start(out=rep, in_=rep_in)
                kv = kvp.tile([128, PG], f16)
                nc.vector.tensor_tensor(out=kv, in0=rep, in1=kd, op=mult)
                nc.tensor.matmul(psb, lhsT=wd_sb[:, p, :], rhs=kv,
                                 start=(p == 0), stop=(p == 31))
            nc.vector.tensor_copy(out=b_sb[:, sl], in_=psb)

        # ---------------- scan step ---------------------------------
        def emit_step(gt):
            col = gt * NSUB
            bsl = b_sb[:, col:col + NSUB]
            if gt == 0:
                nc.vector.tensor_copy(out=cs_all[:, 0, :], in_=bsl)
                return
            cprev = cs_all[:, gt - 1, :]
            _fence(gt_sb[0:1, 0, 0:1])
            for p in range(32):
                mm = nc.tensor.matmul(psU[:, p, :], lhsT=wu_sb[:, p, :],
                                      rhs=cprev, start=True, stop=True)
                if p == 0:
                    STEP_MAP[mm.ins.name] = (0, gt)
            # gate: u *= alpha (broadcast over the 16 chunk-halves)
            a_sl = adup[:, col:col + NSUB]
            for h, eng in ((0, nc.vector), (1, nc.gpsimd)):
                a_b = bass.AP(tensor=a_sl.tensor, offset=a_sl.offset,
                              ap=[a_sl.ap[0], [0, 16], a_sl.ap[1]])
                eng.tensor_tensor(out=gt_sb[:, 16 * h:16 * h + 16, :],
                                  in0=psU[:, 16 * h:16 * h + 16, :],
                                  in1=a_b, op=mult)
            _fence(gt_sb[0:1, 0, 0:1])
            _fence(gt_sb[0:1, 16, 0:1])
            for p in range(32):
                nc.tensor.matmul(psC, lhsT=wd_sb[:, p, :],
                                 rhs=gt_sb[:, p, :],
                                 start=(p == 0), stop=(p == 31))
            eng = nc.gpsimd if gt % 2 == 0 else nc.vector
            eng.tensor_tensor(out=cs_all[:, gt, :], in0=psC, in1=bsl, op=add)

        # ---------------- readout (lane pair w) ---------------------
        def emit_readout(w):
            s = 2 * w
            # q window (contiguous copy), then PE transpose
            qw = opool.tile([64, 128], f16, name="qw")
            qsrc = qkT[0:64, WARM * NSUB + s:WARM * NSUB + s + 1]
            nc.vector.tensor_copy(
                out=qw,
                in_=bass.AP(tensor=qsrc.tensor, offset=qsrc.offset,
                            ap=[qsrc.ap[0], [NSUB, LREAL], [1, 2]]))
            pst = pp.tile([128, 512], f32, name="ps")
            pst16 = pst.bitcast(f16)[:, 0:64]
            _fence(ident[0:1, 0:1])
            nc.tensor.transpose(out=pst16, in_=qw, identity=ident[0:64, 0:64])
            qsb = q_sb[w % 2]
            nc.scalar.copy(out=qsb, in_=pst16)

            csw = cs_all[:, WARM:WIN, s:s + 2]    # (128, 64, 2)
            _fence(qsb[0:1, 0:1])
            oa_p = [opool.tile([128, 64], f32, name=f"oa{i}") for i in range(4)]
            for sl8 in range(8):
                psu = pb.tile([128, PG], f32, name="psb")
                nc.tensor.matmul(psu[:, 0:PG], lhsT=csw,
                                 rhs=wukm_sb[:, sl8 * 512:sl8 * 512 + 256],
                                 start=True, stop=True)
                psu2 = pb.tile([128, PG], f32, name="psb")
                nc.tensor.matmul(psu2[:, 0:PG], lhsT=csw,
                                 rhs=wukm_sb[:, sl8 * 512 + 256:sl8 * 512 + 512],
                                 start=True, stop=True)
                usb = usbp.tile([128, 512], f32, name="usb")
                nc.scalar.copy(out=usb[:, 0:256], in_=psu)
                nc.scalar.copy(out=usb[:, 256:512], in_=psu2)
                acc = oa_p[sl8 % 4]
                for jl in range(8):
                    j = sl8 * 8 + jl
                    qcol = qsb[:, j:j + 1]
                    eng = nc.gpsimd if (sl8 % 2 == 0) else nc.vector
                    if sl8 < 4 and jl == 0:
                        eng.tensor_scalar_mul(
                            out=acc, in0=usb[:, jl * 64:(jl + 1) * 64],
                            scalar1=qcol)
                    else:
                        eng.scalar_tensor_tensor(
                            out=acc, in0=usb[:, jl * 64:(jl + 1) * 64],
                            scalar=qcol, in1=acc, op0=mult, op1=add)
            nc.vector.tensor_tensor(out=oa_p[0], in0=oa_p[0], in1=oa_p[1],
                                    op=add)
            nc.gpsimd.tensor_tensor(out=oa_p[2], in0=oa_p[2], in1=oa_p[3],
                                    op=add)
            oa = opool.tile([128, 64], f32, name="oafin")
            nc.vector.tensor_tensor(out=oa, in0=oa_p[0], in1=oa_p[2], op=add)
            # rows of oa are (t, j) interleaved; scatter to o_out rows
            # (s+j)*64 + t
            dst = o_out[s * 64:s * 64 + 128, :]
            out_ap = bass.AP(tensor=dst.tensor, offset=dst.offset,
                             ap=[[64, 64], [64 * 64, 2], [1, 64]])
            nc.sync.dma_start(out=out_ap, in_=oa)

        # ================= schedule =================
        load_xs(0)
        load_xs(1)
        emit_p1(0)
        _fence(wu_sb[0:1, 0, 0:1])
        p1_done = 1
        for gt in range(WIN):
            # keep P1 one group ahead of the scan
            need_grp = min(NPG - 1, gt // PGT + 1)
            while p1_done <= need_grp:
                if p1_done + 1 < NPG and (p1_done + 1) not in xs_tiles:
                    load_xs(p1_done + 1)
                emit_p1(p1_done)
                p1_done += 1
            emit_step(gt)
        for w in range(NPAIR):
            emit_readout(w)


def _host_prep(inputs):
    x = np.asarray(inputs["x"], np.float32)
    Wq = np.asarray(inputs["W_q"], np.float32)
    Wk = np.asarray(inputs["W_k"], np.float32)
    Wv = np.asarray(inputs["W_v"], np.float32)
    Wa = np.asarray(inputs["W_alpha"], np.float32)
    bq = np.asarray(inputs["b_q"], np.float32)
    bk = np.asarray(inputs["b_k"], np.float32)
    bv = np.asarray(inputs["b_v"], np.float32)
    ba = np.asarray(inputs["b_alpha"], np.float32)
    Wd = np.asarray(inputs["W_down"], np.float32)    # (128, 4096) kv k-major
    Wu = np.asarray(inputs["W_up"], np.float32)      # (4096, 128)
    x0 = np.linalg.lstsq(Wk.astype(np.float64), -bk.astype(np.float64),
                         rcond=None)[0].astype(np.float32)

    # v-major chunk permutation: chunk p, row r -> k=r%64, v=2p+r//64;
    # flat kv index = k*64+v
    p_idx = np.arange(32)
    r_idx = np.arange(128)
    k = np.broadcast_to((r_idx % 64)[None, :], (32, 128))
    v = 2 * p_idx[:, None] + (r_idx // 64)[None, :]
    idx = (k * 64 + v).reshape(-1)                   # (4096,)

    shared = {
        "wqk": np.ascontiguousarray(np.concatenate([Wq, Wk], 0).T),
        "wav": np.ascontiguousarray(np.concatenate([Wa, Wv], 0).T),
        "bqk": np.concatenate([bq, bk]).reshape(128, 1),
        "bav": np.concatenate([ba, bv]).reshape(128, 1),
        "wdvm": np.ascontiguousarray(Wd.T[idx]).astype(np.float16),
        "wuvm": np.ascontiguousarray(Wu.T[:, idx]).astype(np.float16),
        "wukm": np.ascontiguousarray(Wu.T).astype(np.float16),
    }
    in_maps = []
    for core in range(8):
        b, h = core // 2, core % 2
        base = h * 1024
        lanes = []
        for i in range(NSUB):
            lo = base + i * LREAL - WARM
            hi = base + i * LREAL + LREAL
            if lo < 0:
                seg = np.concatenate([np.tile(x0, (-lo, 1)), x[b, 0:hi]],
                                     axis=0)
            else:
                seg = x[b, lo:hi]
            lanes.append(seg)
        xs = np.stack(lanes, axis=1).reshape(TOK, DM)   # token-major
        m = dict(shared)
        m["xsT"] = np.ascontiguousarray(xs.T)
        in_maps.append(m)
    return in_maps


def kernel(**inputs):
    if "nc" not in _BUILT:
        _BUILT["nc"] = _build_bass()
    nc = _BUILT["nc"]
    in_maps = _host_prep(inputs)
    res = run_bass_kernel_spmd(nc, in_maps, core_ids=list(range(8)))
    results = res.results if hasattr(res, "results") else res
    o = np.zeros((B, S, DV), np.float32)
    for core in range(8):
        b, h = core // 2, core % 2
        o[b, h * 1024:(h + 1) * 1024, :] = results[core]["o_out"]
    return o


# revision 41
# speedup vs baseline: 1.8797x; 1.0970x over previous
"""CompressedGLAHead Trainium2 kernel (v3: factored token-major scan).

Math (per batch element b, sequence of S tokens):
  q,k,v,alpha = Linear(x);  alpha = sigmoid(...)
  b_t  = Wd @ vec(k_t v_t^T)
  c_t  = A_t c_{t-1} + b_t,  A_t = Wd diag(rep(alpha_t)) Wu
  o_t  = q_t^T reshape(Wu c_t, (64,64))

Key structure:
  * The A_t matvec is FACTORED per step: u = Wu c (32 matmuls), gate
    u *= alpha (one DVE + one Pool half), c' = Wd gate(u) + b (32
    accumulating matmuls + one engine add).  No per-token A matrices are
    materialized, eliminating the PSUM->SBUF drain traffic that
    dominates an A-materialized design.
  * 16 scan lanes (32 warmup + 64 real tokens each; contractive scan =>
    zero-init + 32-token warmup reproduces the true state; tokens before
    t=0 use a synthetic x0 with W_k x0 + b_k = 0 so the warm state stays
    exactly 0).  The lane streams are interleaved TOKEN-MAJOR: stream
    column gt*16+s holds lane s's token gt.  All 16 lanes advance in one
    step: matmul rhs/state-write/gate operands are all contiguous
    (128, 16) slices.
  * kv chunks are v-major with 4-row v replication (16 rep DMAs per
    projection group) and k duplicated by engine copies; Wd/Wu are
    host-permuted to match.
  * Readout (after the scan): per lane-pair window, psu = cs^T Wu on
    PE, Act drains, q-weighted reduction as 8 parallel stt chains + a
    pairwise add tree on DVE/Pool.
"""

import numpy as np

import concourse.bass as bass
import concourse.tile as tile
from concourse import bacc
from concourse import mybir
from concourse.bass_utils import run_bass_kernel_spmd
from concourse.masks import make_identity

B, S, DM, DK, DV, DC = 4, 2048, 1024, 64, 64, 128
WARM = 32
LREAL = 64
NSUB = 16
WIN = WARM + LREAL          # 96
TOK = NSUB * WIN            # 1536
PGT = 16                    # projection group: 16 token-steps = 256 cols
PG = PGT * NSUB             # 256
NPG = TOK // PG             # 6
NPAIR = NSUB // 2           # 8 readout pair-windows

f32 = mybir.dt.float32
f32r = mybir.dt.float32r
f16 = mybir.dt.float16

_BUILT = {}
STEP_MAP = {}


def _build_bass():
    nc = bacc.Bacc("TRN2", target_bir_lowering=False, debug=False)

    xsT = nc.dram_tensor("xsT", [DM, TOK], f32r, kind="ExternalInput")
    wqk = nc.dram_tensor("wqk", [DM, 128], f32r, kind="ExternalInput")
    wav = nc.dram_tensor("wav", [DM, 128], f32r, kind="ExternalInput")
    bqk = nc.dram_tensor("bqk", [128, 1], f32, kind="ExternalInput")
    bav = nc.dram_tensor("bav", [128, 1], f32, kind="ExternalInput")
    # v-major-permuted Wd/Wu for the scan; k-major Wu for the readout
    wdvm = nc.dram_tensor("wdvm", [DK * DV, DC], f16, kind="ExternalInput")
    wuvm = nc.dram_tensor("wuvm", [DC, DK * DV], f16, kind="ExternalInput")
    wukm = nc.dram_tensor("wukm", [DC, DK * DV], f16, kind="ExternalInput")
    o_out = nc.dram_tensor("o_out", [NSUB * LREAL, DV], f32, kind="ExternalOutput")

    with tile.TileContext(nc) as tc:
        _emit(nc, tc, xsT, wqk, wav, bqk, bav, wdvm, wuvm, wukm, o_out)
    nc.compile()
    return nc


def _emit(nc, tc, xsT, wqk, wav, bqk, bav, wdvm, wuvm, wukm, o_out):
    from contextlib import ExitStack

    add = mybir.AluOpType.add
    mult = mybir.AluOpType.mult
    ACT = mybir.ActivationFunctionType

    def _fence(ap):
        if ap.dtype == f32:
            ap = ap.bitcast(f16)
        nc.tensor.ldweights(weights=ap)

    ctx = ExitStack()
    with ctx:
        consts = ctx.enter_context(tc.tile_pool(name="consts", bufs=1))

        # ---- resident weights ----
        w_qk = consts.tile([128, 8, 128], f32r, name="w_qk")
        src = wqk[:, :]
        nc.sync.dma_start(
            out=w_qk,
            in_=bass.AP(tensor=src.tensor, offset=src.offset,
                        ap=[[128, 128], [128 * 128, 8], [1, 128]]))
        w_av = consts.tile([128, 8, 128], f32r, name="w_av")
        src = wav[:, :]
        nc.sync.dma_start(
            out=w_av,
            in_=bass.AP(tensor=src.tensor, offset=src.offset,
                        ap=[[128, 128], [128 * 128, 8], [1, 128]]))
        b_qk = consts.tile([128, 1], f32, name="b_qk")
        nc.sync.dma_start(out=b_qk, in_=bqk[:, :])
        b_av = consts.tile([128, 1], f32, name="b_av")
        nc.sync.dma_start(out=b_av, in_=bav[:, :])
        wd_sb = consts.tile([128, 32, 128], f16, name="wd_sb")
        srcd = wdvm[:, :]
        nc.sync.dma_start(
            out=wd_sb,
            in_=bass.AP(tensor=srcd.tensor, offset=srcd.offset,
                        ap=[[128, 128], [128 * 128, 32], [1, 128]]))
        wu_sb = consts.tile([128, 32, 128], f16, name="wu_sb")
        nc.sync.dma_start(out=wu_sb, in_=wuvm[:, :])
        wukm_sb = consts.tile([128, 4096], f16, name="wukm_sb")
        nc.sync.dma_start(out=wukm_sb, in_=wukm[:, :])
        ident = consts.tile([128, 128], f16)
        make_identity(nc, ident)

        # ---- persistent activations ----
        qkT = consts.tile([128, TOK], f16)    # rows 0-63 q, 64-127 k
        vaT = consts.tile([128, TOK], f16)    # rows 0-63 alpha, 64-127 v
        adup = consts.tile([128, TOK], f16)   # [alpha; alpha]
        b_sb = consts.tile([128, TOK], f16)   # b_inT (c, col)
        cs_all = consts.tile([128, WIN, NSUB], f16, name="cs_all")
        gt_sb = consts.tile([128, 32, NSUB], f16, name="gt_sb")  # gated u
        q_sb = [consts.tile([128, 64], f32, name=f"q_sb{i}") for i in range(2)]

        xp = ctx.enter_context(tc.tile_pool(name="xp", bufs=2))
        pp = ctx.enter_context(tc.tile_pool(name="pp", bufs=2, space="PSUM"))
        pb = ctx.enter_context(tc.tile_pool(name="pb", bufs=2, space="PSUM"))
        pu = ctx.enter_context(tc.tile_pool(name="pu", bufs=1, space="PSUM"))
        pcp = ctx.enter_context(tc.tile_pool(name="pcp", bufs=1, space="PSUM"))
        kvp = ctx.enter_context(tc.tile_pool(name="kvp", bufs=2))
        repp = ctx.enter_context(tc.tile_pool(name="repp", bufs=4))
        dupp = ctx.enter_context(tc.tile_pool(name="dupp", bufs=2))
        usbp = ctx.enter_context(tc.tile_pool(name="usbp", bufs=2))
        opool = ctx.enter_context(tc.tile_pool(name="opool", bufs=4))

        psU = pu.tile([128, 32, NSUB], f32, name="psU")
        psC = pcp.tile([128, NSUB], f32, name="psC")

        xs_tiles = {}

        def load_xs(p):
            t = xp.tile([128, 8, PG], f32r, name="xs")
            src = xsT[:, :]
            nc.sync.dma_start(
                out=t,
                in_=bass.AP(tensor=src.tensor, offset=src.offset + p * PG,
                            ap=[[TOK, 128], [128 * TOK, 8], [1, PG]]))
            xs_tiles[p] = t

        # ---------------- P1 group: proj + kv + b_in ----------------
        def emit_p1(g):
            sl = slice(g * PG, (g + 1) * PG)
            xs = xs_tiles.pop(g)
            ps = pp.tile([128, 512], f32, name="ps")
            ps_qk, ps_va = ps[:, 0:PG], ps[:, PG:2 * PG]
            if g > 0:
                _fence(qkT[0:1, g * PG - 1:g * PG])
            for s8 in range(8):
                nc.tensor.matmul(ps_qk, lhsT=w_qk[:, s8, :],
                                 rhs=xs[:, s8, :],
                                 start=(s8 == 0), stop=(s8 == 7))
            for s8 in range(8):
                nc.tensor.matmul(ps_va, lhsT=w_av[:, s8, :],
                                 rhs=xs[:, s8, :],
                                 start=(s8 == 0), stop=(s8 == 7))
            nc.scalar.activation(out=qkT[:, sl], in_=ps_qk,
                                 func=ACT.Identity, bias=b_qk)
            nc.scalar.activation(out=vaT[0:64, sl], in_=ps_va[0:64, :],
                                 func=ACT.Sigmoid, bias=b_av[0:64, :])
            nc.scalar.activation(out=vaT[64:128, sl], in_=ps_va[64:128, :],
                                 func=ACT.Identity, bias=b_av[64:128, :])
            # alpha duplicated for the gate ops
            nc.vector.tensor_copy(out=adup[0:64, sl], in_=vaT[0:64, sl])
            nc.gpsimd.tensor_copy(out=adup[64:128, sl], in_=vaT[0:64, sl])

            # kdup: [k; k]
            kd = dupp.tile([128, PG], f16, name="kd")
            nc.gpsimd.tensor_copy(out=kd[0:64, :], in_=qkT[64:128, sl])
            nc.vector.tensor_copy(out=kd[64:128, :], in_=qkT[64:128, sl])
            psb = pb.tile([128, PG], f32, name="psb")
            if g == 0:
                _fence(wd_sb[0:1, 0, 0:1])
            if g >= 2:
                _fence(b_sb[0:1, (g - 2) * PG:(g - 2) * PG + 1])
            # v-major chunk p: rows r -> k=r%64, v=2p+r//64
            for p in range(32):
                rep = repp.tile([128, PG], f16)
                srcv = vaT[64 + 2 * p:64 + 2 * p + 2, sl]
                rep_in = bass.AP(tensor=srcv.tensor, offset=srcv.offset,
                                 ap=[srcv.ap[0], [0, 64]] + list(srcv.ap[1:]))
                nc.sync.dma_start(out=rep, in_=rep_in)
                kv = kvp.tile([128, PG], f16)
                nc.vector.tensor_tensor(out=kv, in0=rep, in1=kd, op=mult)
                nc.tensor.matmul(psb, lhsT=wd_sb[:, p, :], rhs=kv,
                                 start=(p == 0), stop=(p == 31))
            nc.vector.tensor_copy(out=b_sb[:, sl], in_=psb)

        # ---------------- scan step ---------------------------------
        def emit_step(gt):
            col = gt * NSUB
            bsl = b_sb[:, col:col + NSUB]
            if gt == 0:
                nc.vector.tensor_copy(out=cs_all[:, 0, :], in_=bsl)
                return
            cprev = cs_all[:, gt - 1, :]
            _fence(gt_sb[0:1, 0, 0:1])
            for p in range(32):
                mm = nc.tensor.matmul(psU[:, p, :], lhsT=wu_sb[:, p, :],
                                      rhs=cprev, start=True, stop=True)
                if p == 0:
                    STEP_MAP[mm.ins.name] = (0, gt)
            # gate: u *= alpha (broadcast over the 32 chunks; DVE only —
            # GPSIMD cannot access PSUM on real hardware)
            a_sl = adup[:, col:col + NSUB]
            a_b = bass.AP(tensor=a_sl.tensor, offset=a_sl.offset,
                          ap=[a_sl.ap[0], [0, 32], a_sl.ap[1]])
            nc.vector.tensor_tensor(out=gt_sb, in0=psU, in1=a_b, op=mult)
            _fence(gt_sb[0:1, 0, 0:1])
            _fence(gt_sb[0:1, 16, 0:1])
            for p in range(32):
                nc.tensor.matmul(psC, lhsT=wd_sb[:, p, :],
                                 rhs=gt_sb[:, p, :],
                                 start=(p == 0), stop=(p == 31))
            nc.vector.tensor_tensor(out=cs_all[:, gt, :], in0=psC, in1=bsl,
                                    op=add)

        # ---------------- readout (lane pair w) ---------------------
        def emit_readout(w):
            s = 2 * w
            # q window (contiguous copy), then PE transpose
            qw = opool.tile([64, 128], f16, name="qw")
            qsrc = qkT[0:64, WARM * NSUB + s:WARM * NSUB + s + 1]
            nc.vector.tensor_copy(
                out=qw,
                in_=bass.AP(tensor=qsrc.tensor, offset=qsrc.offset,
                            ap=[qsrc.ap[0], [NSUB, LREAL], [1, 2]]))
            pst = pp.tile([128, 512], f32, name="ps")
            pst16 = pst.bitcast(f16)[:, 0:64]
            _fence(ident[0:1, 0:1])
            nc.tensor.transpose(out=pst16, in_=qw, identity=ident[0:64, 0:64])
            qsb = q_sb[w % 2]
            nc.scalar.copy(out=qsb, in_=pst16)

            csw = cs_all[:, WARM:WIN, s:s + 2]    # (128, 64, 2)
            _fence(qsb[0:1, 0:1])
            oa_p = [opool.tile([128, 64], f32, name=f"oa{i}") for i in range(4)]
            for sl8 in range(8):
                psu = pb.tile([128, PG], f32, name="psb")
                nc.tensor.matmul(psu[:, 0:PG], lhsT=csw,
                                 rhs=wukm_sb[:, sl8 * 512:sl8 * 512 + 256],
                                 start=True, stop=True)
                psu2 = pb.tile([128, PG], f32, name="psb")
                nc.tensor.matmul(psu2[:, 0:PG], lhsT=csw,
                                 rhs=wukm_sb[:, sl8 * 512 + 256:sl8 * 512 + 512],
                                 start=True, stop=True)
                usb = usbp.tile([128, 512], f32, name="usb")
                nc.scalar.copy(out=usb[:, 0:256], in_=psu)
                nc.scalar.copy(out=usb[:, 256:512], in_=psu2)
                acc = oa_p[sl8 % 4]
                for jl in range(8):
                    j = sl8 * 8 + jl
                    qcol = qsb[:, j:j + 1]
                    eng = nc.gpsimd if (sl8 % 2 == 0) else nc.vector
                    if sl8 < 4 and jl == 0:
                        eng.tensor_scalar_mul(
                            out=acc, in0=usb[:, jl * 64:(jl + 1) * 64],
                            scalar1=qcol)
                    else:
                        eng.scalar_tensor_tensor(
                            out=acc, in0=usb[:, jl * 64:(jl + 1) * 64],
                            scalar=qcol, in1=acc, op0=mult, op1=add)
            nc.vector.tensor_tensor(out=oa_p[0], in0=oa_p[0], in1=oa_p[1],
                                    op=add)
            nc.gpsimd.tensor_tensor(out=oa_p[2], in0=oa_p[2], in1=oa_p[3],
                                    op=add)
            oa = opool.tile([128, 64], f32, name="oafin")
            nc.vector.tensor_tensor(out=oa, in0=oa_p[0], in1=oa_p[2], op=add)
            # rows of oa are (t, j) interleaved; scatter to o_out rows
            # (s+j)*64 + t
            dst = o_out[s * 64:s * 64 + 128, :]
            out_ap = bass.AP(tensor=dst.tensor, offset=dst.offset,
                             ap=[[64, 64], [64 * 64, 2], [1, 64]])
            nc.sync.dma_start(out=out_ap, in_=oa)

        # ================= schedule =================
        load_xs(0)
        load_xs(1)
        emit_p1(0)
        _fence(wu_sb[0:1, 0, 0:1])
        p1_done = 1
        for gt in range(WIN):
            # keep P1 one group ahead of the scan
            need_grp = min(NPG - 1, gt // PGT + 1)
            while p1_done <= need_grp:
                if p1_done + 1 < NPG and (p1_done + 1) not in xs_tiles:
                    load_xs(p1_done + 1)
                emit_p1(p1_done)
                p1_done += 1
            emit_step(gt)
        for w in range(NPAIR):
            emit_readout(w)


def _host_prep(inputs):
    x = np.asarray(inputs["x"], np.float32)
    Wq = np.asarray(inputs["W_q"], np.float32)
    Wk = np.asarray(inputs["W_k"], np.float32)
    Wv = np.asarray(inputs["W_v"], np.float32)
    Wa = np.asarray(inputs["W_alpha"], np.float32)
    bq = np.asarray(inputs["b_q"], np.float32)
    bk = np.asarray(inputs["b_k"], np.float32)
    bv = np.asarray(inputs["b_v"], np.float32)
    ba = np.asarray(inputs["b_alpha"], np.float32)
    Wd = np.asarray(inputs["W_down"], np.float32)    # (128, 4096) kv k-major
    Wu = np.asarray(inputs["W_up"], np.float32)      # (4096, 128)
    x0 = np.linalg.lstsq(Wk.astype(np.float64), -bk.astype(np.float64),
                         rcond=None)[0].astype(np.float32)

    # v-major chunk permutation: chunk p, row r -> k=r%64, v=2p+r//64;
    # flat kv index = k*64+v
    p_idx = np.arange(32)
    r_idx = np.arange(128)
    k = np.broadcast_to((r_idx % 64)[None, :], (32, 128))
    v = 2 * p_idx[:, None] + (r_idx // 64)[None, :]
    idx = (k * 64 + v).reshape(-1)                   # (4096,)

    shared = {
        "wqk": np.ascontiguousarray(np.concatenate([Wq, Wk], 0).T),
        "wav": np.ascontiguousarray(np.concatenate([Wa, Wv], 0).T),
        "bqk": np.concatenate([bq, bk]).reshape(128, 1),
        "bav": np.concatenate([ba, bv]).reshape(128, 1),
        "wdvm": np.ascontiguousarray(Wd.T[idx]).astype(np.float16),
        "wuvm": np.ascontiguousarray(Wu.T[:, idx]).astype(np.float16),
        "wukm": np.ascontiguousarray(Wu.T).astype(np.float16),
    }
    in_maps = []
    for core in range(8):
        b, h = core // 2, core % 2
        base = h * 1024
        lanes = []
        for i in range(NSUB):
            lo = base + i * LREAL - WARM
            hi = base + i * LREAL + LREAL
            if lo < 0:
                seg = np.concatenate([np.tile(x0, (-lo, 1)), x[b, 0:hi]],
                                     axis=0)
            else:
                seg = x[b, lo:hi]
            lanes.append(seg)
        xs = np.stack(lanes, axis=1).reshape(TOK, DM)   # token-major
        m = dict(shared)
        m["xsT"] = np.ascontiguousarray(xs.T)
        in_maps.append(m)
    return in_maps


def kernel(**inputs):
    if "nc" not in _BUILT:
        _BUILT["nc"] = _build_bass()
    nc = _BUILT["nc"]
    in_maps = _host_prep(inputs)
    res = run_bass_kernel_spmd(nc, in_maps, core_ids=list(range(8)))
    results = res.results if hasattr(res, "results") else res
    o = np.zeros((B, S, DV), np.float32)
    for core in range(8):
        b, h = core // 2, core % 2
        o[b, h * 1024:(h + 1) * 1024, :] = results[core]["o_out"]
    return o
